# revision 6
# baseline (speedup 1.0000x reference)
"""CFConvS2V Trainium2 kernel (8-core data-parallel over batch), v3.

reference computation:
    h = silu(layernorm(s @ W1.T + b1))               # (B, N, H)
    v[b,i,c,d] = sum_j mask[b,i,j] * ev[b,i,j,c] * h[b,j,d]   # (B, N, 3, H)

Sharding: data-parallel over B across 8 cores (4 batches each); the pairwise
tensors and the j-reduction stay local per core.

v3 design (memory-roofline targeted):
  * All big tensors staged in fp16 (host-side cast): halves HBM traffic vs
    fp32. Element rounding ~5e-4 relative, far inside the 2e-2 gate.
  * ev staged TRANSPOSED on the host to [b, j, (c, i)] and concatenated with
    the transposed mask [b, j, i] into one [BL, N, 2048] tensor: one
    contiguous 512KB DMA per (batch, j-chunk), j lands on partitions, so the
    contraction needs NO on-chip transposes.
  * mask applied by one DVE multiply per j-chunk (fp16 keeps DVE in 2x mode);
    this is the dominant DVE cost (~1us x 16) and sets the DVE floor.
  * contraction: per (b, jc): 4 accumulating fp16 matmuls with h[jc]
    stationary into 4 PSUM accumulators (one per i-tile, 384 cols each).
  * h-phase engineered off the DVE:
      - bias via K=1 rank-1 matmuls (ones (x) b1) seeding PSUM,
      - row means via an extra matmul against w1rs = rowsum(W1T)/H into 4
        extra PSUM columns,
      - E[x^2] via ACT Square(x/sqrt(H)) with accum_out,
      - normalize+SiLU fused into one ACT op per chunk:
        Silu(x*rstd - mu*rstd) with per-partition scale/bias APs.
    DVE h-work drops to 4 small ops per batch.
  * reps loop unrolled 3x per For_i iteration (each iteration carries an
    all-engine barrier; unrolling amortizes the pipeline drain/refill).
  * output evicted to fp16, stored as [d, (it, c, il)]; host reorders and
    upcasts. Total HBM traffic/core ~10.6 MB -> ~30us roofline at 358 GB/s.
"""

import sys

if "/opt/trn_rl_repo" not in sys.path:
    sys.path.insert(0, "/opt/trn_rl_repo")

from contextlib import ExitStack

import numpy as np

import concourse.bass as bass
import concourse.mybir as mybir
from concourse.tile import TileContext

B, N, H, C = 32, 512, 128, 3
NCORES = 8
BL = B // NCORES      # batches per core
P = 128
NT = N // P           # i-tiles per batch
JC = N // P           # j-chunks
EVW = C * N + N       # ev row (1536) + mask row (512) per (b, j)
LN_EPS = 1e-5
F32 = mybir.dt.float32
F16 = mybir.dt.float16
AF = mybir.ActivationFunctionType
ALU = mybir.AluOpType
UNROLL = 3


def _split_multi_waits(nc):
    """The walrus build in this container only accepts one sync-wait per
    instruction; hoist extra waits onto single-wait NOPs in front."""
    ctr = 0
    for f in nc.m.functions:
        for bb in f.blocks:
            insts = bb.instructions
            i = 0
            while i < len(insts):
                inst = insts[i]
                si = inst.sync_info
                if si is not None and len(si.on_wait) > 1:
                    waits = list(si.on_wait)
                    for w in waits[:-1]:
                        ctr += 1
                        nop = mybir.InstNoOp(
                            name=f"splitwait-{ctr}",
                            engine=inst.engine,
                            sync_info=mybir.SyncInfo(on_wait=[w], on_update=[]),
                            bass_nofuse=True,
                        )
                        nc.register_instruction(nop, overwrite=True)
                        insts.insert(i, nop)
                        i += 1
                    inst.sync_info = mybir.SyncInfo(
                        on_wait=[waits[-1]], on_update=list(si.on_update)
                    )
                i += 1


def build(reps=1):
    nc = bass.Bass("TRN2", target_bir_lowering=False, debug=False, num_devices=NCORES)
    evm = nc.dram_tensor("evm", [BL, N, EVW], F16, kind="ExternalInput").ap()
    sT = nc.dram_tensor("sT", [BL, H, N], F16, kind="ExternalInput").ap()
    w1t = nc.dram_tensor("w1t", [H, H], F16, kind="ExternalInput").ap()
    w1rs = nc.dram_tensor("w1rs", [H, 1], F16, kind="ExternalInput").ap()
    # cst row: [ones(H) | tile(b1, NT) | mean(b1) x NT] for the K=1 matmuls
    cst = nc.dram_tensor("cst", [1, H + N + NT], F16, kind="ExternalInput").ap()
    out = nc.dram_tensor("out", [BL, H, NT * C * P], F16, kind="ExternalOutput").ap()

    with TileContext(nc) as tc, ExitStack() as ctx:
        const = ctx.enter_context(tc.tile_pool(name="const", bufs=1))
        p_sT = ctx.enter_context(tc.tile_pool(name="p_sT", bufs=2))
        p_h = ctx.enter_context(tc.tile_pool(name="p_h", bufs=2))
        p_stat = ctx.enter_context(tc.tile_pool(name="p_stat", bufs=4))
        p_evm = ctx.enter_context(tc.tile_pool(name="p_evm", bufs=8))
        p_mev = ctx.enter_context(tc.tile_pool(name="p_mev", bufs=6))
        p_vout = ctx.enter_context(tc.tile_pool(name="p_vout", bufs=3))
        ps_h = ctx.enter_context(tc.tile_pool(name="ps_h", bufs=1, space="PSUM"))
        ps_v = ctx.enter_context(tc.tile_pool(name="ps_v", bufs=6, space="PSUM"))

        w1t_sb = const.tile([H, H], F16)
        nc.sync.dma_start(out=w1t_sb[:], in_=w1t[:])
        w1rs_sb = const.tile([H, 1], F16)
        nc.sync.dma_start(out=w1rs_sb[:], in_=w1rs[:])
        cst_sb = const.tile([1, H + N + NT], F16)
        nc.sync.dma_start(out=cst_sb[:], in_=cst[:])
        eps_sb = const.tile([P, 1], F32)
        nc.vector.memset(eps_sb[:], LN_EPS)

        def h_phase(b, sT_sb, h_sb):
            # x = s @ W1.T + b1 into psum cols [0:512]; row-means into
            # cols [512:516] via the w1rs column (+ mean(b1) seed).
            psum_h = ps_h.tile([P, 1024], F32)
            nc.tensor.matmul(
                out=psum_h[:, 0:N],
                lhsT=cst_sb[:, 0:H],
                rhs=cst_sb[:, H : H + N],
                start=True,
                stop=False,
                skip_group_check=True,
            )
            nc.tensor.matmul(
                out=psum_h[:, N : N + NT],
                lhsT=cst_sb[:, 0:H],
                rhs=cst_sb[:, H + N : H + N + NT],
                start=True,
                stop=False,
                skip_group_check=True,
            )
            for t in range(NT):
                nc.tensor.matmul(
                    out=psum_h[:, t * P : (t + 1) * P],
                    lhsT=sT_sb[:, b, t * P : (t + 1) * P],
                    rhs=w1t_sb[:],
                    start=False,
                    stop=False,
                    skip_group_check=True,
                )
                nc.tensor.matmul(
                    out=psum_h[:, N + t : N + t + 1],
                    lhsT=sT_sb[:, b, t * P : (t + 1) * P],
                    rhs=w1rs_sb[:],
                    start=False,
                    stop=(t == NT - 1),
                    skip_group_check=True,
                )
            # E[x^2] per chunk: Square(x/sqrt(H)) summed along free dim
            sq = p_stat.tile([P, H], F32, tag="sq")
            ex2 = p_stat.tile([P, NT], F32, tag="ex2")
            for t in range(NT):
                nc.scalar.activation(
                    out=sq[:],
                    in_=psum_h[:, t * P : (t + 1) * P],
                    func=AF.Square,
                    scale=float(1.0 / np.sqrt(H)),
                    accum_out=ex2[:, t : t + 1],
                )
            mu = psum_h[:, N : N + NT]
            var = p_stat.tile([P, NT], F32, tag="var")
            nc.scalar.activation(out=var[:], in_=mu, func=AF.Square)
            nc.vector.tensor_sub(out=var[:], in0=ex2[:], in1=var[:])
            rstd = p_stat.tile([P, NT], F32, tag="rstd")
            nc.scalar.activation(
                out=rstd[:], in_=var[:], func=AF.Sqrt, bias=eps_sb[:]
            )
            nc.vector.reciprocal(out=rstd[:], in_=rstd[:])
            negmr = p_stat.tile([P, NT], F32, tag="negmr")
            nc.vector.scalar_tensor_tensor(
                out=negmr[:],
                in0=mu,
                scalar=-1.0,
                in1=rstd[:],
                op0=ALU.mult,
                op1=ALU.mult,
            )
            # h = Silu(x*rstd - mu*rstd), straight from PSUM, emitted fp16
            for t in range(NT):
                nc.scalar.activation(
                    out=h_sb[:, b, t * P : (t + 1) * P],
                    in_=psum_h[:, t * P : (t + 1) * P],
                    func=AF.Silu,
                    bias=negmr[:, t : t + 1],
                    scale=rstd[:, t : t + 1],
                )

        def main_phase(b, h_sb):
            psvs = [
                ps_v.tile([P, 512], F32, name=f"psv{it}", tag="psv")
                for it in range(NT)
            ]
            for jc in range(JC):
                evm_sb = p_evm.tile([P, EVW], F16)
                nc.sync.dma_start(out=evm_sb[:], in_=evm[b, jc * P : (jc + 1) * P])
                # mev[j,(c,i)] = ev[j,(c,i)] * mask[j,i]  (broadcast over c)
                mev = p_mev.tile([P, C, N], F16)
                nc.vector.tensor_tensor(
                    out=mev[:],
                    in0=evm_sb[:, : C * N].rearrange("p (c i) -> p c i", i=N),
                    in1=evm_sb[:, C * N :].unsqueeze(1).broadcast_to((P, C, N)),
                    op=ALU.mult,
                )
                for it in range(NT):
                    # v[d, (c,il)] += sum_j h[j, d] * mev[j, (c, il)]
                    nc.tensor.matmul(
                        out=psvs[it][:, : C * P],
                        lhsT=h_sb[:, b, jc * P : (jc + 1) * P],
                        rhs=mev[:, :, it * P : (it + 1) * P],
                        start=(jc == 0),
                        stop=(jc == JC - 1),
                        skip_group_check=True,
                    )
            vout = p_vout.tile([P, NT, C * P], F16)
            for it in range(NT):
                nc.scalar.activation(
                    out=vout[:, it, :], in_=psvs[it][:, : C * P], func=AF.Copy
                )
            # store on the ACT HWDGE ring so stores can't block loads on
            # the SP-ring FIFO
            nc.scalar.dma_start(
                out=out[b], in_=vout[:].rearrange("p t f -> p (t f)")
            )

        def body():
            sT_sb = p_sT.tile([P, BL, N], F16)
            for b in range(BL):
                nc.sync.dma_start(out=sT_sb[:, b, :], in_=sT[b])
            h_sb = p_h.tile([P, BL, N], F16)
            for b in range(BL):
                h_phase(b, sT_sb, h_sb)
                main_phase(b, h_sb)

        n_loop, n_rem = divmod(reps, UNROLL)
        if n_loop > 0:
            with tc.For_i(0, n_loop, 1):
                for _ in range(UNROLL):
                    body()
        for _ in range(n_rem):
            body()

    _split_multi_waits(nc)
    return nc


_built_nc = None


def _get_nc():
    global _built_nc
    if _built_nc is None:
        _built_nc = build()
    return _built_nc


def shard_inputs(s, ev, mask, W1, b1):
    """Full inputs -> list of per-core input dicts (fp16 staged layouts)."""
    s = np.asarray(s, dtype=np.float32)
    ev = np.asarray(ev, dtype=np.float32)
    mask = np.asarray(mask, dtype=np.float32)
    W1 = np.asarray(W1, dtype=np.float32)
    b1 = np.asarray(b1, dtype=np.float32)
    w1t = np.ascontiguousarray(W1.T).astype(np.float16)
    w1rs = (W1.sum(axis=0) / H).reshape(H, 1).astype(np.float16)
    cst = np.concatenate(
        [
            np.ones((1, H), np.float32),
            np.tile(b1[None, :], (1, NT)),
            np.full((1, NT), float(b1.mean()), np.float32),
        ],
        axis=1,
    ).astype(np.float16)
    in_maps = []
    for m in range(NCORES):
        bs = slice(m * BL, (m + 1) * BL)
        evt = ev[bs].transpose(0, 2, 3, 1).reshape(BL, N, C * N)  # [b, j, (c,i)]
        mst = mask[bs, :, :, 0].transpose(0, 2, 1)                # [b, j, i]
        evm = np.concatenate([evt, mst], axis=2).astype(np.float16)
        in_maps.append(
            {
                "evm": np.ascontiguousarray(evm),
                "sT": np.ascontiguousarray(s[bs].transpose(0, 2, 1)).astype(
                    np.float16
                ),
                "w1t": w1t,
                "w1rs": w1rs,
                "cst": cst,
            }
        )
    return in_maps


def unshard_output(per_core_outs):
    """list of per-core "out" arrays [BL, H, NT*C*P] fp16 -> full (B, N, 3, H)."""
    parts = []
    for o in per_core_outs:
        o = o.astype(np.float32).reshape(BL, H, NT, C, P).transpose(0, 2, 4, 3, 1)
        parts.append(np.ascontiguousarray(o).reshape(BL, N, C, H))
    return np.concatenate(parts, axis=0)


_executor = None


def _get_executor():
    """Build the sharded PJRT executable once; reuse across kernel() calls."""
    global _executor
    if _executor is not None:
        return _executor
    import jax
    from jax.sharding import Mesh, PartitionSpec
    from jax.experimental.shard_map import shard_map

    from concourse import bass2jax

    bass2jax.install_neuronx_cc_hook()
    nc = _get_nc()
    partition_name = nc.partition_id_tensor.name if nc.partition_id_tensor else None
    in_names, out_names, out_avals, zero_outs = [], [], [], []
    for alloc in nc.m.functions[0].allocations:
        if not isinstance(alloc, mybir.MemoryLocationSet):
            continue
        name = alloc.memorylocations[0].name
        if alloc.kind == "ExternalInput":
            if name != partition_name:
                in_names.append(name)
        elif alloc.kind == "ExternalOutput":
            out_names.append(name)
            shape = tuple(alloc.tensor_shape)
            dtype = mybir.dt.np(alloc.dtype)
            out_avals.append(jax.core.ShapedArray(shape, dtype))
            zero_outs.append(np.zeros(shape, dtype))
    n_params = len(in_names)
    all_in_names = list(in_names) + list(out_names)
    if partition_name is not None:
        all_in_names.append(partition_name)

    def _body(*args):
        operands = list(args)
        if partition_name is not None:
            operands.append(bass2jax.partition_id_tensor())
        outs = bass2jax._bass_exec_p.bind(
            *operands,
            out_avals=tuple(out_avals),
            in_names=tuple(all_in_names),
            out_names=tuple(out_names),
            lowering_input_output_aliases=(),
            sim_require_finite=True,
            sim_require_nnan=True,
            nc=nc,
        )
        return tuple(outs)

    devices = jax.devices()[:NCORES]
    mesh = Mesh(np.asarray(devices), ("core",))
    donate = tuple(range(n_params, n_params + len(out_names)))
    fn = jax.jit(
        shard_map(
            _body,
            mesh=mesh,
            in_specs=(PartitionSpec("core"),) * (n_params + len(out_names)),
            out_specs=(PartitionSpec("core"),) * len(out_names),
            check_rep=False,
        ),
        donate_argnums=donate,
        keep_unused=True,
    )
    _executor = (fn, in_names, out_names, out_avals, zero_outs)
    return _executor


def kernel(s, ev, mask, W1, b1):
    fn, in_names, out_names, out_avals, zero_outs = _get_executor()
    in_maps = shard_inputs(s, ev, mask, W1, b1)
    concat_in = [
        np.concatenate([in_maps[c][nm] for c in range(NCORES)], axis=0)
        for nm in in_names
    ]
    concat_zeros = [
        np.zeros((NCORES * z.shape[0], *z.shape[1:]), z.dtype) for z in zero_outs
    ]
    out_arrs = fn(*concat_in, *concat_zeros)
    i = out_names.index("out")
    o = np.asarray(out_arrs[i]).reshape(NCORES, *out_avals[i].shape)
    return unshard_output([o[c] for c in range(NCORES)])


# revision 10
# speedup vs baseline: 1.0085x; 1.0085x over previous
"""CFConvS2V Trainium2 kernel (8-core data-parallel over batch), v3.

reference computation:
    h = silu(layernorm(s @ W1.T + b1))               # (B, N, H)
    v[b,i,c,d] = sum_j mask[b,i,j] * ev[b,i,j,c] * h[b,j,d]   # (B, N, 3, H)

Sharding: data-parallel over B across 8 cores (4 batches each); the pairwise
tensors and the j-reduction stay local per core.

v3 design (memory-roofline targeted):
  * All big tensors staged in fp16 (host-side cast): halves HBM traffic vs
    fp32. Element rounding ~5e-4 relative, far inside the 2e-2 gate.
  * ev staged TRANSPOSED on the host to [b, j, (c, i)] and concatenated with
    the transposed mask [b, j, i] into one [BL, N, 2048] tensor: one
    contiguous 512KB DMA per (batch, j-chunk), j lands on partitions, so the
    contraction needs NO on-chip transposes.
  * mask applied by one DVE multiply per j-chunk (fp16 keeps DVE in 2x mode);
    this is the dominant DVE cost (~1us x 16) and sets the DVE floor.
  * contraction: per (b, jc): 4 accumulating fp16 matmuls with h[jc]
    stationary into 4 PSUM accumulators (one per i-tile, 384 cols each).
  * h-phase engineered off the DVE:
      - bias via K=1 rank-1 matmuls (ones (x) b1) seeding PSUM,
      - row means via an extra matmul against w1rs = rowsum(W1T)/H into 4
        extra PSUM columns,
      - E[x^2] via ACT Square(x/sqrt(H)) with accum_out,
      - normalize+SiLU fused into one ACT op per chunk:
        Silu(x*rstd - mu*rstd) with per-partition scale/bias APs.
    DVE h-work drops to 4 small ops per batch.
  * reps loop unrolled 3x per For_i iteration (each iteration carries an
    all-engine barrier; unrolling amortizes the pipeline drain/refill).
  * output evicted to fp16, stored as [d, (it, c, il)]; host reorders and
    upcasts. Total HBM traffic/core ~10.6 MB -> ~30us roofline at 358 GB/s.
"""

import sys

if "/opt/trn_rl_repo" not in sys.path:
    sys.path.insert(0, "/opt/trn_rl_repo")

from contextlib import ExitStack

import numpy as np

import concourse.bass as bass
import concourse.mybir as mybir
from concourse.tile import TileContext

B, N, H, C = 32, 512, 128, 3
NCORES = 8
BL = B // NCORES      # batches per core
P = 128
NT = N // P           # i-tiles per batch
JC = N // P           # j-chunks
EVW = C * N + N       # ev row (1536) + mask row (512) per (b, j)
LN_EPS = 1e-5
F32 = mybir.dt.float32
F16 = mybir.dt.float16
AF = mybir.ActivationFunctionType
ALU = mybir.AluOpType
# --- tunables (sweepable via sweep.py; defaults are the shipping config) ---
UNROLL = 1            # bodies per For_i iteration (amortizes the barrier)
PS_H_BUFS = 1
PS_V_BUFS = 6
P_EVM_BUFS = 8
P_MEV_BUFS = 6
STAGGERED = False     # For_i staggered_reset
MODE = "full"         # "full" | "dma_only" (timing diagnostic) | "no_dve"


def _split_multi_waits(nc):
    """The walrus build in this container only accepts one sync-wait per
    instruction; hoist extra waits onto single-wait NOPs in front."""
    ctr = 0
    for f in nc.m.functions:
        for bb in f.blocks:
            insts = bb.instructions
            i = 0
            while i < len(insts):
                inst = insts[i]
                si = inst.sync_info
                if si is not None and len(si.on_wait) > 1:
                    waits = list(si.on_wait)
                    for w in waits[:-1]:
                        ctr += 1
                        nop = mybir.InstNoOp(
                            name=f"splitwait-{ctr}",
                            engine=inst.engine,
                            sync_info=mybir.SyncInfo(on_wait=[w], on_update=[]),
                            bass_nofuse=True,
                        )
                        nc.register_instruction(nop, overwrite=True)
                        insts.insert(i, nop)
                        i += 1
                    inst.sync_info = mybir.SyncInfo(
                        on_wait=[waits[-1]], on_update=list(si.on_update)
                    )
                i += 1


def build(reps=1):
    nc = bass.Bass("TRN2", target_bir_lowering=False, debug=False, num_devices=NCORES)
    evm = nc.dram_tensor("evm", [BL, N, EVW], F16, kind="ExternalInput").ap()
    sT = nc.dram_tensor("sT", [BL, H, N], F16, kind="ExternalInput").ap()
    w1t = nc.dram_tensor("w1t", [H, H], F16, kind="ExternalInput").ap()
    w1rs = nc.dram_tensor("w1rs", [H, 1], F16, kind="ExternalInput").ap()
    # cst row: [ones(H) | tile(b1, NT) | mean(b1) x NT] for the K=1 matmuls
    cst = nc.dram_tensor("cst", [1, H + N + NT], F16, kind="ExternalInput").ap()
    out = nc.dram_tensor("out", [BL, H, NT * C * P], F16, kind="ExternalOutput").ap()

    with TileContext(nc) as tc, ExitStack() as ctx:
        const = ctx.enter_context(tc.tile_pool(name="const", bufs=1))
        p_sT = ctx.enter_context(tc.tile_pool(name="p_sT", bufs=2))
        p_h = ctx.enter_context(tc.tile_pool(name="p_h", bufs=2))
        p_stat = ctx.enter_context(tc.tile_pool(name="p_stat", bufs=4))
        p_evm = ctx.enter_context(tc.tile_pool(name="p_evm", bufs=P_EVM_BUFS))
        p_mev = ctx.enter_context(tc.tile_pool(name="p_mev", bufs=P_MEV_BUFS))
        p_vout = ctx.enter_context(tc.tile_pool(name="p_vout", bufs=3))
        ps_h = ctx.enter_context(tc.tile_pool(name="ps_h", bufs=PS_H_BUFS, space="PSUM"))
        ps_v = ctx.enter_context(tc.tile_pool(name="ps_v", bufs=PS_V_BUFS, space="PSUM"))

        w1t_sb = const.tile([H, H], F16)
        nc.sync.dma_start(out=w1t_sb[:], in_=w1t[:])
        w1rs_sb = const.tile([H, 1], F16)
        nc.sync.dma_start(out=w1rs_sb[:], in_=w1rs[:])
        cst_sb = const.tile([1, H + N + NT], F16)
        nc.sync.dma_start(out=cst_sb[:], in_=cst[:])
        eps_sb = const.tile([P, 1], F32)
        nc.vector.memset(eps_sb[:], LN_EPS)
        if MODE == "dma_only":
            vz_c = const.tile([P, NT * C * P], F16)
            nc.vector.memset(vz_c[:].bitcast(F32), 0.0)

        def h_phase(b, sT_sb, h_sb):
            # x = s @ W1.T + b1 into psum cols [0:512]; row-means into
            # cols [512:516] via the w1rs column (+ mean(b1) seed).
            psum_h = ps_h.tile([P, 1024], F32)
            nc.tensor.matmul(
                out=psum_h[:, 0:N],
                lhsT=cst_sb[:, 0:H],
                rhs=cst_sb[:, H : H + N],
                start=True,
                stop=False,
                skip_group_check=True,
            )
            nc.tensor.matmul(
                out=psum_h[:, N : N + NT],
                lhsT=cst_sb[:, 0:H],
                rhs=cst_sb[:, H + N : H + N + NT],
                start=True,
                stop=False,
                skip_group_check=True,
            )
            for t in range(NT):
                nc.tensor.matmul(
                    out=psum_h[:, t * P : (t + 1) * P],
                    lhsT=sT_sb[:, b, t * P : (t + 1) * P],
                    rhs=w1t_sb[:],
                    start=False,
                    stop=False,
                    skip_group_check=True,
                )
                nc.tensor.matmul(
                    out=psum_h[:, N + t : N + t + 1],
                    lhsT=sT_sb[:, b, t * P : (t + 1) * P],
                    rhs=w1rs_sb[:],
                    start=False,
                    stop=(t == NT - 1),
                    skip_group_check=True,
                )
            # E[x^2] per chunk: Square(x/sqrt(H)) summed along free dim
            sq = p_stat.tile([P, H], F32, tag="sq")
            ex2 = p_stat.tile([P, NT], F32, tag="ex2")
            for t in range(NT):
                nc.scalar.activation(
                    out=sq[:],
                    in_=psum_h[:, t * P : (t + 1) * P],
                    func=AF.Square,
                    scale=float(1.0 / np.sqrt(H)),
                    accum_out=ex2[:, t : t + 1],
                )
            mu = psum_h[:, N : N + NT]
            var = p_stat.tile([P, NT], F32, tag="var")
            nc.scalar.activation(out=var[:], in_=mu, func=AF.Square)
            nc.vector.tensor_sub(out=var[:], in0=ex2[:], in1=var[:])
            rstd = p_stat.tile([P, NT], F32, tag="rstd")
            nc.scalar.activation(
                out=rstd[:], in_=var[:], func=AF.Sqrt, bias=eps_sb[:]
            )
            nc.vector.reciprocal(out=rstd[:], in_=rstd[:])
            negmr = p_stat.tile([P, NT], F32, tag="negmr")
            nc.vector.scalar_tensor_tensor(
                out=negmr[:],
                in0=mu,
                scalar=-1.0,
                in1=rstd[:],
                op0=ALU.mult,
                op1=ALU.mult,
            )
            # h = Silu(x*rstd - mu*rstd), straight from PSUM, emitted fp16
            for t in range(NT):
                nc.scalar.activation(
                    out=h_sb[:, b, t * P : (t + 1) * P],
                    in_=psum_h[:, t * P : (t + 1) * P],
                    func=AF.Silu,
                    bias=negmr[:, t : t + 1],
                    scale=rstd[:, t : t + 1],
                )

        def main_phase(b, h_sb):
            psvs = [
                ps_v.tile([P, 512], F32, name=f"psv{it}", tag="psv")
                for it in range(NT)
            ]
            for jc in range(JC):
                evm_sb = p_evm.tile([P, EVW], F16)
                nc.sync.dma_start(out=evm_sb[:], in_=evm[b, jc * P : (jc + 1) * P])
                # mev[j,(c,i)] = ev[j,(c,i)] * mask[j,i]  (broadcast over c)
                if MODE == "no_dve":
                    mev = evm_sb[:, : C * N].rearrange("p (c i) -> p c i", i=N)
                else:
                    mev_t = p_mev.tile([P, C, N], F16)
                    nc.vector.tensor_tensor(
                        out=mev_t[:],
                        in0=evm_sb[:, : C * N].rearrange("p (c i) -> p c i", i=N),
                        in1=evm_sb[:, C * N :].unsqueeze(1).broadcast_to((P, C, N)),
                        op=ALU.mult,
                    )
                    mev = mev_t[:]
                for it in range(NT):
                    # v[d, (c,il)] += sum_j h[j, d] * mev[j, (c, il)]
                    nc.tensor.matmul(
                        out=psvs[it][:, : C * P],
                        lhsT=h_sb[:, b, jc * P : (jc + 1) * P],
                        rhs=mev[:, :, it * P : (it + 1) * P],
                        start=(jc == 0),
                        stop=(jc == JC - 1),
                        skip_group_check=True,
                    )
            vout = p_vout.tile([P, NT, C * P], F16)
            for it in range(NT):
                nc.scalar.activation(
                    out=vout[:, it, :], in_=psvs[it][:, : C * P], func=AF.Copy
                )
            # store on the ACT HWDGE ring so stores can't block loads on
            # the SP-ring FIFO
            nc.scalar.dma_start(
                out=out[b], in_=vout[:].rearrange("p t f -> p (t f)")
            )

        def body():
            sT_sb = p_sT.tile([P, BL, N], F16)
            for b in range(BL):
                nc.sync.dma_start(out=sT_sb[:, b, :], in_=sT[b])
            if MODE == "dma_only":
                for b in range(BL):
                    for jc in range(JC):
                        evm_sb = p_evm.tile([P, EVW], F16)
                        nc.sync.dma_start(
                            out=evm_sb[:], in_=evm[b, jc * P : (jc + 1) * P]
                        )
                    nc.scalar.dma_start(out=out[b], in_=vz_c[:])
                return
            h_sb = p_h.tile([P, BL, N], F16)
            for b in range(BL):
                h_phase(b, sT_sb, h_sb)
                main_phase(b, h_sb)

        n_loop, n_rem = divmod(reps, UNROLL)
        if n_loop > 0:
            with tc.For_i(0, n_loop, 1, staggered_reset=STAGGERED):
                for _ in range(UNROLL):
                    body()
        for _ in range(n_rem):
            body()

    _split_multi_waits(nc)
    return nc


_built_nc = None


def _get_nc():
    global _built_nc
    if _built_nc is None:
        _built_nc = build()
    return _built_nc


def shard_inputs(s, ev, mask, W1, b1):
    """Full inputs -> list of per-core input dicts (fp16 staged layouts)."""
    s = np.asarray(s, dtype=np.float32)
    ev = np.asarray(ev, dtype=np.float32)
    mask = np.asarray(mask, dtype=np.float32)
    W1 = np.asarray(W1, dtype=np.float32)
    b1 = np.asarray(b1, dtype=np.float32)
    w1t = np.ascontiguousarray(W1.T).astype(np.float16)
    w1rs = (W1.sum(axis=0) / H).reshape(H, 1).astype(np.float16)
    cst = np.concatenate(
        [
            np.ones((1, H), np.float32),
            np.tile(b1[None, :], (1, NT)),
            np.full((1, NT), float(b1.mean()), np.float32),
        ],
        axis=1,
    ).astype(np.float16)
    in_maps = []
    for m in range(NCORES):
        bs = slice(m * BL, (m + 1) * BL)
        evt = ev[bs].transpose(0, 2, 3, 1).reshape(BL, N, C * N)  # [b, j, (c,i)]
        mst = mask[bs, :, :, 0].transpose(0, 2, 1)                # [b, j, i]
        evm = np.concatenate([evt, mst], axis=2).astype(np.float16)
        in_maps.append(
            {
                "evm": np.ascontiguousarray(evm),
                "sT": np.ascontiguousarray(s[bs].transpose(0, 2, 1)).astype(
                    np.float16
                ),
                "w1t": w1t,
                "w1rs": w1rs,
                "cst": cst,
            }
        )
    return in_maps


def unshard_output(per_core_outs):
    """list of per-core "out" arrays [BL, H, NT*C*P] fp16 -> full (B, N, 3, H)."""
    parts = []
    for o in per_core_outs:
        o = o.astype(np.float32).reshape(BL, H, NT, C, P).transpose(0, 2, 4, 3, 1)
        parts.append(np.ascontiguousarray(o).reshape(BL, N, C, H))
    return np.concatenate(parts, axis=0)


_executor = None


def _get_executor():
    """Build the sharded PJRT executable once; reuse across kernel() calls."""
    global _executor
    if _executor is not None:
        return _executor
    import jax
    from jax.sharding import Mesh, PartitionSpec
    from jax.experimental.shard_map import shard_map

    from concourse import bass2jax

    bass2jax.install_neuronx_cc_hook()
    nc = _get_nc()
    partition_name = nc.partition_id_tensor.name if nc.partition_id_tensor else None
    in_names, out_names, out_avals, zero_outs = [], [], [], []
    for alloc in nc.m.functions[0].allocations:
        if not isinstance(alloc, mybir.MemoryLocationSet):
            continue
        name = alloc.memorylocations[0].name
        if alloc.kind == "ExternalInput":
            if name != partition_name:
                in_names.append(name)
        elif alloc.kind == "ExternalOutput":
            out_names.append(name)
            shape = tuple(alloc.tensor_shape)
            dtype = mybir.dt.np(alloc.dtype)
            out_avals.append(jax.core.ShapedArray(shape, dtype))
            zero_outs.append(np.zeros(shape, dtype))
    n_params = len(in_names)
    all_in_names = list(in_names) + list(out_names)
    if partition_name is not None:
        all_in_names.append(partition_name)

    def _body(*args):
        operands = list(args)
        if partition_name is not None:
            operands.append(bass2jax.partition_id_tensor())
        outs = bass2jax._bass_exec_p.bind(
            *operands,
            out_avals=tuple(out_avals),
            in_names=tuple(all_in_names),
            out_names=tuple(out_names),
            lowering_input_output_aliases=(),
            sim_require_finite=True,
            sim_require_nnan=True,
            nc=nc,
        )
        return tuple(outs)

    devices = jax.devices()[:NCORES]
    mesh = Mesh(np.asarray(devices), ("core",))
    donate = tuple(range(n_params, n_params + len(out_names)))
    fn = jax.jit(
        shard_map(
            _body,
            mesh=mesh,
            in_specs=(PartitionSpec("core"),) * (n_params + len(out_names)),
            out_specs=(PartitionSpec("core"),) * len(out_names),
            check_rep=False,
        ),
        donate_argnums=donate,
        keep_unused=True,
    )
    _executor = (fn, in_names, out_names, out_avals, zero_outs)
    return _executor


def kernel(s, ev, mask, W1, b1):
    fn, in_names, out_names, out_avals, zero_outs = _get_executor()
    in_maps = shard_inputs(s, ev, mask, W1, b1)
    concat_in = [
        np.concatenate([in_maps[c][nm] for c in range(NCORES)], axis=0)
        for nm in in_names
    ]
    concat_zeros = [
        np.zeros((NCORES * z.shape[0], *z.shape[1:]), z.dtype) for z in zero_outs
    ]
    out_arrs = fn(*concat_in, *concat_zeros)
    i = out_names.index("out")
    o = np.asarray(out_arrs[i]).reshape(NCORES, *out_avals[i].shape)
    return unshard_output([o[c] for c in range(NCORES)])


# revision 13
# speedup vs baseline: 1.4734x; 1.4609x over previous
"""CFConvS2V Trainium2 kernel (8-core data-parallel over batch), v3.

reference computation:
    h = silu(layernorm(s @ W1.T + b1))               # (B, N, H)
    v[b,i,c,d] = sum_j mask[b,i,j] * ev[b,i,j,c] * h[b,j,d]   # (B, N, 3, H)

Sharding: data-parallel over B across 8 cores (4 batches each); the pairwise
tensors and the j-reduction stay local per core.

v3 design (memory-roofline targeted):
  * All big tensors staged in fp16 (host-side cast): halves HBM traffic vs
    fp32. Element rounding ~5e-4 relative, far inside the 2e-2 gate.
  * ev staged TRANSPOSED on the host to [b, j, (c, i)] and concatenated with
    the transposed mask [b, j, i] into one [BL, N, 2048] tensor: one
    contiguous 512KB DMA per (batch, j-chunk), j lands on partitions, so the
    contraction needs NO on-chip transposes.
  * mask applied by one DVE multiply per j-chunk (fp16 keeps DVE in 2x mode);
    this is the dominant DVE cost (~1us x 16) and sets the DVE floor.
  * contraction: per (b, jc): 4 accumulating fp16 matmuls with h[jc]
    stationary into 4 PSUM accumulators (one per i-tile, 384 cols each).
  * h-phase engineered off the DVE:
      - bias via K=1 rank-1 matmuls (ones (x) b1) seeding PSUM,
      - row means via an extra matmul against w1rs = rowsum(W1T)/H into 4
        extra PSUM columns,
      - E[x^2] via ACT Square(x/sqrt(H)) with accum_out,
      - normalize+SiLU fused into one ACT op per chunk:
        Silu(x*rstd - mu*rstd) with per-partition scale/bias APs.
    DVE h-work drops to 4 small ops per batch.
  * reps loop unrolled 3x per For_i iteration (each iteration carries an
    all-engine barrier; unrolling amortizes the pipeline drain/refill).
  * output evicted to fp16, stored as [d, (it, c, il)]; host reorders and
    upcasts. Total HBM traffic/core ~10.6 MB -> ~30us roofline at 358 GB/s.
"""

import sys

if "/opt/trn_rl_repo" not in sys.path:
    sys.path.insert(0, "/opt/trn_rl_repo")

from contextlib import ExitStack

import numpy as np

import concourse.bass as bass
import concourse.mybir as mybir
from concourse.tile import TileContext

B, N, H, C = 32, 512, 128, 3
NCORES = 8
BL = B // NCORES      # batches per core
P = 128
NT = N // P           # i-tiles per batch
JC = N // P           # j-chunks
EVW = C * N + N       # ev row (1536) + mask row (512) per (b, j)
LN_EPS = 1e-5
F32 = mybir.dt.float32
F16 = mybir.dt.float16
AF = mybir.ActivationFunctionType
ALU = mybir.AluOpType
# --- tunables (sweepable via sweep.py; defaults are the shipping config) ---
UNROLL = 3            # bodies per For_i iteration (amortizes the barrier)
H_AHEAD = 0           # batches the h-phase runs ahead of the main phase
RECHUNK = True        # contraction in 3x512-wide MMs (c-planes) vs 4x384
PS_H_BUFS = 2
PS_V_BUFS = 4
P_EVM_BUFS = 8
P_MEV_BUFS = 6
STAGGERED = False     # For_i staggered_reset
MODE = "full"         # "full" | "dma_only" (timing diagnostic) | "no_dve"


def _split_multi_waits(nc):
    """The walrus build in this container only accepts one sync-wait per
    instruction; hoist extra waits onto single-wait NOPs in front."""
    ctr = 0
    for f in nc.m.functions:
        for bb in f.blocks:
            insts = bb.instructions
            i = 0
            while i < len(insts):
                inst = insts[i]
                si = inst.sync_info
                if si is not None and len(si.on_wait) > 1:
                    waits = list(si.on_wait)
                    for w in waits[:-1]:
                        ctr += 1
                        nop = mybir.InstNoOp(
                            name=f"splitwait-{ctr}",
                            engine=inst.engine,
                            sync_info=mybir.SyncInfo(on_wait=[w], on_update=[]),
                            bass_nofuse=True,
                        )
                        nc.register_instruction(nop, overwrite=True)
                        insts.insert(i, nop)
                        i += 1
                    inst.sync_info = mybir.SyncInfo(
                        on_wait=[waits[-1]], on_update=list(si.on_update)
                    )
                i += 1


def build(reps=1):
    nc = bass.Bass("TRN2", target_bir_lowering=False, debug=False, num_devices=NCORES)
    evm = nc.dram_tensor("evm", [BL, N, EVW], F16, kind="ExternalInput").ap()
    sT = nc.dram_tensor("sT", [BL, H, N], F16, kind="ExternalInput").ap()
    w1t = nc.dram_tensor("w1t", [H, H], F16, kind="ExternalInput").ap()
    w1rs = nc.dram_tensor("w1rs", [H, 1], F16, kind="ExternalInput").ap()
    # cst row: [ones(H) | tile(b1, NT) | mean(b1) x NT] for the K=1 matmuls
    cst = nc.dram_tensor("cst", [1, H + N + NT], F16, kind="ExternalInput").ap()
    out = nc.dram_tensor("out", [BL, H, NT * C * P], F16, kind="ExternalOutput").ap()

    with TileContext(nc) as tc, ExitStack() as ctx:
        const = ctx.enter_context(tc.tile_pool(name="const", bufs=1))
        p_sT = ctx.enter_context(tc.tile_pool(name="p_sT", bufs=2))
        p_h = ctx.enter_context(tc.tile_pool(name="p_h", bufs=2))
        p_stat = ctx.enter_context(tc.tile_pool(name="p_stat", bufs=4))
        p_evm = ctx.enter_context(tc.tile_pool(name="p_evm", bufs=P_EVM_BUFS))
        p_mev = ctx.enter_context(tc.tile_pool(name="p_mev", bufs=P_MEV_BUFS))
        p_vout = ctx.enter_context(tc.tile_pool(name="p_vout", bufs=3))
        ps_h = ctx.enter_context(tc.tile_pool(name="ps_h", bufs=PS_H_BUFS, space="PSUM"))
        ps_v = ctx.enter_context(tc.tile_pool(name="ps_v", bufs=PS_V_BUFS, space="PSUM"))

        w1t_sb = const.tile([H, H], F16)
        nc.sync.dma_start(out=w1t_sb[:], in_=w1t[:])
        w1rs_sb = const.tile([H, 1], F16)
        nc.sync.dma_start(out=w1rs_sb[:], in_=w1rs[:])
        cst_sb = const.tile([1, H + N + NT], F16)
        nc.sync.dma_start(out=cst_sb[:], in_=cst[:])
        eps_sb = const.tile([P, 1], F32)
        nc.vector.memset(eps_sb[:], LN_EPS)
        if MODE == "dma_only":
            vz_c = const.tile([P, NT * C * P], F16)
            nc.vector.memset(vz_c[:].bitcast(F32), 0.0)

        def h_phase(b, sT_sb, h_sb):
            # x = s @ W1.T + b1 into psum cols [0:512]; row-means into
            # cols [512:516] via the w1rs column (+ mean(b1) seed).
            psum_h = ps_h.tile([P, 1024], F32)
            nc.tensor.matmul(
                out=psum_h[:, 0:N],
                lhsT=cst_sb[:, 0:H],
                rhs=cst_sb[:, H : H + N],
                start=True,
                stop=False,
                skip_group_check=True,
            )
            nc.tensor.matmul(
                out=psum_h[:, N : N + NT],
                lhsT=cst_sb[:, 0:H],
                rhs=cst_sb[:, H + N : H + N + NT],
                start=True,
                stop=False,
                skip_group_check=True,
            )
            for t in range(NT):
                nc.tensor.matmul(
                    out=psum_h[:, t * P : (t + 1) * P],
                    lhsT=sT_sb[:, b, t * P : (t + 1) * P],
                    rhs=w1t_sb[:],
                    start=False,
                    stop=False,
                    skip_group_check=True,
                )
                nc.tensor.matmul(
                    out=psum_h[:, N + t : N + t + 1],
                    lhsT=sT_sb[:, b, t * P : (t + 1) * P],
                    rhs=w1rs_sb[:],
                    start=False,
                    stop=(t == NT - 1),
                    skip_group_check=True,
                )
            # E[x^2] per chunk: Square(x/sqrt(H)) summed along free dim
            sq = p_stat.tile([P, H], F32, tag="sq")
            ex2 = p_stat.tile([P, NT], F32, tag="ex2")
            for t in range(NT):
                nc.scalar.activation(
                    out=sq[:],
                    in_=psum_h[:, t * P : (t + 1) * P],
                    func=AF.Square,
                    scale=float(1.0 / np.sqrt(H)),
                    accum_out=ex2[:, t : t + 1],
                )
            mu = psum_h[:, N : N + NT]
            var = p_stat.tile([P, NT], F32, tag="var")
            nc.scalar.activation(out=var[:], in_=mu, func=AF.Square)
            nc.vector.tensor_sub(out=var[:], in0=ex2[:], in1=var[:])
            rstd = p_stat.tile([P, NT], F32, tag="rstd")
            nc.scalar.activation(
                out=rstd[:], in_=var[:], func=AF.Sqrt, bias=eps_sb[:]
            )
            nc.vector.reciprocal(out=rstd[:], in_=rstd[:])
            negmr = p_stat.tile([P, NT], F32, tag="negmr")
            nc.vector.scalar_tensor_tensor(
                out=negmr[:],
                in0=mu,
                scalar=-1.0,
                in1=rstd[:],
                op0=ALU.mult,
                op1=ALU.mult,
            )
            # h = Silu(x*rstd - mu*rstd), straight from PSUM, emitted fp16
            for t in range(NT):
                nc.scalar.activation(
                    out=h_sb[:, b, t * P : (t + 1) * P],
                    in_=psum_h[:, t * P : (t + 1) * P],
                    func=AF.Silu,
                    bias=negmr[:, t : t + 1],
                    scale=rstd[:, t : t + 1],
                )

        def main_phase(b, h_sb):
            n_acc = C if RECHUNK else NT
            accw = N if RECHUNK else C * P
            psvs = [
                ps_v.tile([P, 512], F32, name=f"psv{it}", tag="psv")
                for it in range(n_acc)
            ]
            for jc in range(JC):
                evm_sb = p_evm.tile([P, EVW], F16)
                nc.sync.dma_start(out=evm_sb[:], in_=evm[b, jc * P : (jc + 1) * P])
                # mev[j,(c,i)] = ev[j,(c,i)] * mask[j,i]  (broadcast over c)
                if MODE == "no_dve":
                    mev = evm_sb[:, : C * N].rearrange("p (c i) -> p c i", i=N)
                else:
                    mev_t = p_mev.tile([P, C, N], F16)
                    nc.vector.tensor_tensor(
                        out=mev_t[:],
                        in0=evm_sb[:, : C * N].rearrange("p (c i) -> p c i", i=N),
                        in1=evm_sb[:, C * N :].unsqueeze(1).broadcast_to((P, C, N)),
                        op=ALU.mult,
                    )
                    mev = mev_t[:]
                for k in range(n_acc):
                    # v[d, (c,il)] += sum_j h[j, d] * mev[j, (c, il)]
                    if RECHUNK:
                        rhs = mev[:, k, :]          # c-plane, 512 wide
                    else:
                        rhs = mev[:, :, k * P : (k + 1) * P]
                    nc.tensor.matmul(
                        out=psvs[k][:, :accw],
                        lhsT=h_sb[:, b, jc * P : (jc + 1) * P],
                        rhs=rhs,
                        start=(jc == 0),
                        stop=(jc == JC - 1),
                        skip_group_check=True,
                    )
            vout = p_vout.tile([P, n_acc, accw], F16)
            for k in range(n_acc):
                nc.scalar.activation(
                    out=vout[:, k, :], in_=psvs[k][:, :accw], func=AF.Copy
                )
            # store on the ACT HWDGE ring so stores can't block loads on
            # the SP-ring FIFO
            nc.scalar.dma_start(
                out=out[b], in_=vout[:].rearrange("p t f -> p (t f)")
            )

        def body():
            sT_sb = p_sT.tile([P, BL, N], F16)
            for b in range(BL):
                nc.sync.dma_start(out=sT_sb[:, b, :], in_=sT[b])
            if MODE == "dma_only":
                for b in range(BL):
                    for jc in range(JC):
                        evm_sb = p_evm.tile([P, EVW], F16)
                        nc.sync.dma_start(
                            out=evm_sb[:], in_=evm[b, jc * P : (jc + 1) * P]
                        )
                    nc.scalar.dma_start(out=out[b], in_=vz_c[:])
                return
            h_sb = p_h.tile([P, BL, N], F16)
            # software-pipeline: run the h-phase H_AHEAD batches ahead so its
            # long cross-engine LN chain resolves during earlier batches'
            # main phases instead of stalling DVE/PE at each batch boundary
            if H_AHEAD == 0:
                for b in range(BL):
                    h_phase(b, sT_sb, h_sb)
                    main_phase(b, h_sb)
            else:
                for b in range(min(H_AHEAD, BL)):
                    h_phase(b, sT_sb, h_sb)
                for b in range(BL):
                    main_phase(b, h_sb)
                    if b + H_AHEAD < BL:
                        h_phase(b + H_AHEAD, sT_sb, h_sb)

        n_loop, n_rem = divmod(reps, UNROLL)
        if n_loop > 0:
            with tc.For_i(0, n_loop, 1, staggered_reset=STAGGERED):
                for _ in range(UNROLL):
                    body()
        for _ in range(n_rem):
            body()

    _split_multi_waits(nc)
    return nc


_built_nc = None


def _get_nc():
    global _built_nc
    if _built_nc is None:
        _built_nc = build()
    return _built_nc


def shard_inputs(s, ev, mask, W1, b1):
    """Full inputs -> list of per-core input dicts (fp16 staged layouts)."""
    s = np.asarray(s, dtype=np.float32)
    ev = np.asarray(ev, dtype=np.float32)
    mask = np.asarray(mask, dtype=np.float32)
    W1 = np.asarray(W1, dtype=np.float32)
    b1 = np.asarray(b1, dtype=np.float32)
    w1t = np.ascontiguousarray(W1.T).astype(np.float16)
    w1rs = (W1.sum(axis=0) / H).reshape(H, 1).astype(np.float16)
    cst = np.concatenate(
        [
            np.ones((1, H), np.float32),
            np.tile(b1[None, :], (1, NT)),
            np.full((1, NT), float(b1.mean()), np.float32),
        ],
        axis=1,
    ).astype(np.float16)
    in_maps = []
    for m in range(NCORES):
        bs = slice(m * BL, (m + 1) * BL)
        evt = ev[bs].transpose(0, 2, 3, 1).reshape(BL, N, C * N)  # [b, j, (c,i)]
        mst = mask[bs, :, :, 0].transpose(0, 2, 1)                # [b, j, i]
        evm = np.concatenate([evt, mst], axis=2).astype(np.float16)
        in_maps.append(
            {
                "evm": np.ascontiguousarray(evm),
                "sT": np.ascontiguousarray(s[bs].transpose(0, 2, 1)).astype(
                    np.float16
                ),
                "w1t": w1t,
                "w1rs": w1rs,
                "cst": cst,
            }
        )
    return in_maps


def unshard_output(per_core_outs):
    """list of per-core "out" arrays [BL, H, NT*C*P] fp16 -> full (B, N, 3, H)."""
    parts = []
    for o in per_core_outs:
        o = o.astype(np.float32)
        if RECHUNK:
            o = o.reshape(BL, H, C, N).transpose(0, 3, 2, 1)
        else:
            o = o.reshape(BL, H, NT, C, P).transpose(0, 2, 4, 3, 1)
        parts.append(np.ascontiguousarray(o).reshape(BL, N, C, H))
    return np.concatenate(parts, axis=0)


_executor = None


def _get_executor():
    """Build the sharded PJRT executable once; reuse across kernel() calls."""
    global _executor
    if _executor is not None:
        return _executor
    import jax
    from jax.sharding import Mesh, PartitionSpec
    from jax.experimental.shard_map import shard_map

    from concourse import bass2jax

    bass2jax.install_neuronx_cc_hook()
    nc = _get_nc()
    partition_name = nc.partition_id_tensor.name if nc.partition_id_tensor else None
    in_names, out_names, out_avals, zero_outs = [], [], [], []
    for alloc in nc.m.functions[0].allocations:
        if not isinstance(alloc, mybir.MemoryLocationSet):
            continue
        name = alloc.memorylocations[0].name
        if alloc.kind == "ExternalInput":
            if name != partition_name:
                in_names.append(name)
        elif alloc.kind == "ExternalOutput":
            out_names.append(name)
            shape = tuple(alloc.tensor_shape)
            dtype = mybir.dt.np(alloc.dtype)
            out_avals.append(jax.core.ShapedArray(shape, dtype))
            zero_outs.append(np.zeros(shape, dtype))
    n_params = len(in_names)
    all_in_names = list(in_names) + list(out_names)
    if partition_name is not None:
        all_in_names.append(partition_name)

    def _body(*args):
        operands = list(args)
        if partition_name is not None:
            operands.append(bass2jax.partition_id_tensor())
        outs = bass2jax._bass_exec_p.bind(
            *operands,
            out_avals=tuple(out_avals),
            in_names=tuple(all_in_names),
            out_names=tuple(out_names),
            lowering_input_output_aliases=(),
            sim_require_finite=True,
            sim_require_nnan=True,
            nc=nc,
        )
        return tuple(outs)

    devices = jax.devices()[:NCORES]
    mesh = Mesh(np.asarray(devices), ("core",))
    donate = tuple(range(n_params, n_params + len(out_names)))
    fn = jax.jit(
        shard_map(
            _body,
            mesh=mesh,
            in_specs=(PartitionSpec("core"),) * (n_params + len(out_names)),
            out_specs=(PartitionSpec("core"),) * len(out_names),
            check_rep=False,
        ),
        donate_argnums=donate,
        keep_unused=True,
    )
    _executor = (fn, in_names, out_names, out_avals, zero_outs)
    return _executor


def kernel(s, ev, mask, W1, b1):
    fn, in_names, out_names, out_avals, zero_outs = _get_executor()
    in_maps = shard_inputs(s, ev, mask, W1, b1)
    concat_in = [
        np.concatenate([in_maps[c][nm] for c in range(NCORES)], axis=0)
        for nm in in_names
    ]
    concat_zeros = [
        np.zeros((NCORES * z.shape[0], *z.shape[1:]), z.dtype) for z in zero_outs
    ]
    out_arrs = fn(*concat_in, *concat_zeros)
    i = out_names.index("out")
    o = np.asarray(out_arrs[i]).reshape(NCORES, *out_avals[i].shape)
    return unshard_output([o[c] for c in range(NCORES)])


# revision 18
# speedup vs baseline: 1.5730x; 1.0677x over previous
"""CFConvS2V Trainium2 kernel (8-core data-parallel over batch), v3.

reference computation:
    h = silu(layernorm(s @ W1.T + b1))               # (B, N, H)
    v[b,i,c,d] = sum_j mask[b,i,j] * ev[b,i,j,c] * h[b,j,d]   # (B, N, 3, H)

Sharding: data-parallel over B across 8 cores (4 batches each); the pairwise
tensors and the j-reduction stay local per core.

v3 design (memory-roofline targeted):
  * All big tensors staged in fp16 (host-side cast): halves HBM traffic vs
    fp32. Element rounding ~5e-4 relative, far inside the 2e-2 gate.
  * ev staged TRANSPOSED on the host to [b, j, (c, i)] and concatenated with
    the transposed mask [b, j, i] into one [BL, N, 2048] tensor: one
    contiguous 512KB DMA per (batch, j-chunk), j lands on partitions, so the
    contraction needs NO on-chip transposes.
  * mask applied by one DVE multiply per j-chunk (fp16 keeps DVE in 2x mode);
    this is the dominant DVE cost (~1us x 16) and sets the DVE floor.
  * contraction: per (b, jc): 4 accumulating fp16 matmuls with h[jc]
    stationary into 4 PSUM accumulators (one per i-tile, 384 cols each).
  * h-phase engineered off the DVE:
      - bias via K=1 rank-1 matmuls (ones (x) b1) seeding PSUM,
      - row means via an extra matmul against w1rs = rowsum(W1T)/H into 4
        extra PSUM columns,
      - E[x^2] via ACT Square(x/sqrt(H)) with accum_out,
      - normalize+SiLU fused into one ACT op per chunk:
        Silu(x*rstd - mu*rstd) with per-partition scale/bias APs.
    DVE h-work drops to 4 small ops per batch.
  * reps loop unrolled 3x per For_i iteration (each iteration carries an
    all-engine barrier; unrolling amortizes the pipeline drain/refill).
  * output evicted to fp16, stored as [d, (it, c, il)]; host reorders and
    upcasts. Total HBM traffic/core ~10.6 MB -> ~30us roofline at 358 GB/s.
"""

import sys

if "/opt/trn_rl_repo" not in sys.path:
    sys.path.insert(0, "/opt/trn_rl_repo")

from contextlib import ExitStack

import numpy as np

import concourse.bass as bass
import concourse.mybir as mybir
from concourse.tile import TileContext

B, N, H, C = 32, 512, 128, 3
NCORES = 8
BL = B // NCORES      # batches per core
P = 128
NT = N // P           # i-tiles per batch
JC = N // P           # j-chunks
EVW = C * N + N       # ev row (1536) + mask row (512) per (b, j)
LN_EPS = 1e-5
F32 = mybir.dt.float32
F16 = mybir.dt.float16
U8 = mybir.dt.uint8
F8 = mybir.dt.float8e4
AF = mybir.ActivationFunctionType
ALU = mybir.AluOpType
# --- tunables (sweepable via sweep.py; defaults are the shipping config) ---
UNROLL = 9            # bodies per For_i iteration (amortizes the barrier)
H_AHEAD = 0           # batches the h-phase runs ahead of the main phase
RECHUNK = True        # contraction in 3x512-wide MMs (c-planes) vs 4x384
MASK8 = False         # stage mask as fp8 (exact for 0/1), upconvert on GPSIMD
H_STATS = "mm"        # "mm": mean via matmul col (2-bank psum_h); "act": via ACT
                      # Copy+accum (1-bank psum_h, frees banks for ps_v)
PS_H_BUFS = 2
PS_V_BUFS = 4
P_EVM_BUFS = 8
P_MEV_BUFS = 6
STAGGERED = False     # For_i staggered_reset
MODE = "full"         # "full" | "dma_only" (timing diagnostic) | "no_dve"


def _split_multi_waits(nc):
    """The walrus build in this container only accepts one sync-wait per
    instruction; hoist extra waits onto single-wait NOPs in front."""
    ctr = 0
    for f in nc.m.functions:
        for bb in f.blocks:
            insts = bb.instructions
            i = 0
            while i < len(insts):
                inst = insts[i]
                si = inst.sync_info
                if si is not None and len(si.on_wait) > 1:
                    waits = list(si.on_wait)
                    for w in waits[:-1]:
                        ctr += 1
                        nop = mybir.InstNoOp(
                            name=f"splitwait-{ctr}",
                            engine=inst.engine,
                            sync_info=mybir.SyncInfo(on_wait=[w], on_update=[]),
                            bass_nofuse=True,
                        )
                        nc.register_instruction(nop, overwrite=True)
                        insts.insert(i, nop)
                        i += 1
                    inst.sync_info = mybir.SyncInfo(
                        on_wait=[waits[-1]], on_update=list(si.on_update)
                    )
                i += 1


def build(reps=1):
    nc = bass.Bass("TRN2", target_bir_lowering=False, debug=False, num_devices=NCORES)
    if MASK8:
        # bytes: 3072 of fp16 ev + 512 of fp8 mask per (b, j) row
        evm = nc.dram_tensor("evm", [BL, N, 2 * C * N + N], U8,
                             kind="ExternalInput").ap()
    else:
        evm = nc.dram_tensor("evm", [BL, N, EVW], F16, kind="ExternalInput").ap()
    sT = nc.dram_tensor("sT", [BL, H, N], F16, kind="ExternalInput").ap()
    w1t = nc.dram_tensor("w1t", [H, H], F16, kind="ExternalInput").ap()
    w1rs = nc.dram_tensor("w1rs", [H, 1], F16, kind="ExternalInput").ap()
    # cst row: [ones(H) | tile(b1, NT) | mean(b1) x NT] for the K=1 matmuls
    cst = nc.dram_tensor("cst", [1, H + N + NT], F16, kind="ExternalInput").ap()
    out = nc.dram_tensor("out", [BL, H, NT * C * P], F16, kind="ExternalOutput").ap()

    with TileContext(nc) as tc, ExitStack() as ctx:
        const = ctx.enter_context(tc.tile_pool(name="const", bufs=1))
        p_sT = ctx.enter_context(tc.tile_pool(name="p_sT", bufs=2))
        p_h = ctx.enter_context(tc.tile_pool(name="p_h", bufs=2))
        p_stat = ctx.enter_context(tc.tile_pool(name="p_stat", bufs=4))
        p_evm = ctx.enter_context(tc.tile_pool(name="p_evm", bufs=P_EVM_BUFS))
        p_mev = ctx.enter_context(tc.tile_pool(name="p_mev", bufs=P_MEV_BUFS))
        p_mk = ctx.enter_context(tc.tile_pool(name="p_mk", bufs=P_MEV_BUFS))
        p_vout = ctx.enter_context(tc.tile_pool(name="p_vout", bufs=3))
        ps_h = ctx.enter_context(tc.tile_pool(name="ps_h", bufs=PS_H_BUFS, space="PSUM"))
        ps_v = ctx.enter_context(tc.tile_pool(name="ps_v", bufs=PS_V_BUFS, space="PSUM"))

        w1t_sb = const.tile([H, H], F16)
        nc.sync.dma_start(out=w1t_sb[:], in_=w1t[:])
        w1rs_sb = const.tile([H, 1], F16)
        nc.sync.dma_start(out=w1rs_sb[:], in_=w1rs[:])
        cst_sb = const.tile([1, H + N + NT], F16)
        nc.sync.dma_start(out=cst_sb[:], in_=cst[:])
        eps_sb = const.tile([P, 1], F32)
        nc.vector.memset(eps_sb[:], LN_EPS)
        if MODE == "dma_only":
            vz_c = const.tile([P, NT * C * P], F16)
            nc.vector.memset(vz_c[:].bitcast(F32), 0.0)

        def h_phase(b, sT_sb, h_sb):
            # x = s @ W1.T + b1 into psum cols [0:512]; H_STATS="mm" adds
            # row-means into cols [512:516] via the w1rs column.
            psum_h = ps_h.tile([P, 1024 if H_STATS == "mm" else 512], F32)
            nc.tensor.matmul(
                out=psum_h[:, 0:N],
                lhsT=cst_sb[:, 0:H],
                rhs=cst_sb[:, H : H + N],
                start=True,
                stop=False,
                skip_group_check=True,
            )
            if H_STATS == "mm":
                nc.tensor.matmul(
                    out=psum_h[:, N : N + NT],
                    lhsT=cst_sb[:, 0:H],
                    rhs=cst_sb[:, H + N : H + N + NT],
                    start=True,
                    stop=False,
                    skip_group_check=True,
                )
            for t in range(NT):
                nc.tensor.matmul(
                    out=psum_h[:, t * P : (t + 1) * P],
                    lhsT=sT_sb[:, b, t * P : (t + 1) * P],
                    rhs=w1t_sb[:],
                    start=False,
                    stop=(H_STATS != "mm" and t == NT - 1),
                    skip_group_check=True,
                )
                if H_STATS == "mm":
                    nc.tensor.matmul(
                        out=psum_h[:, N + t : N + t + 1],
                        lhsT=sT_sb[:, b, t * P : (t + 1) * P],
                        rhs=w1rs_sb[:],
                        start=False,
                        stop=(t == NT - 1),
                        skip_group_check=True,
                    )
            # E[x^2] per chunk: Square(x/sqrt(H)) summed along free dim
            sq = p_stat.tile([P, H], F32, tag="sq")
            ex2 = p_stat.tile([P, NT], F32, tag="ex2")
            for t in range(NT):
                nc.scalar.activation(
                    out=sq[:],
                    in_=psum_h[:, t * P : (t + 1) * P],
                    func=AF.Square,
                    scale=float(1.0 / np.sqrt(H)),
                    accum_out=ex2[:, t : t + 1],
                )
            if H_STATS == "mm":
                mu = psum_h[:, N : N + NT]
                mu_sbuf = None
            else:
                # sums via ACT Copy+accum (xc is a scratch eviction)
                xc = p_stat.tile([P, H], F32, tag="xc")
                sum4 = p_stat.tile([P, NT], F32, tag="sum4")
                for t in range(NT):
                    nc.scalar.activation(
                        out=xc[:],
                        in_=psum_h[:, t * P : (t + 1) * P],
                        func=AF.Copy,
                        accum_out=sum4[:, t : t + 1],
                    )
                mu = None
                mu_sbuf = sum4
            var = p_stat.tile([P, NT], F32, tag="var")
            if H_STATS == "mm":
                nc.scalar.activation(out=var[:], in_=mu, func=AF.Square)
            else:
                nc.scalar.activation(
                    out=var[:], in_=mu_sbuf[:], func=AF.Square,
                    scale=float(1.0 / H),
                )
            nc.vector.tensor_sub(out=var[:], in0=ex2[:], in1=var[:])
            rstd = p_stat.tile([P, NT], F32, tag="rstd")
            nc.scalar.activation(
                out=rstd[:], in_=var[:], func=AF.Sqrt, bias=eps_sb[:]
            )
            nc.vector.reciprocal(out=rstd[:], in_=rstd[:])
            negmr = p_stat.tile([P, NT], F32, tag="negmr")
            nc.vector.scalar_tensor_tensor(
                out=negmr[:],
                in0=mu if H_STATS == "mm" else mu_sbuf[:],
                scalar=-1.0 if H_STATS == "mm" else -1.0 / H,
                in1=rstd[:],
                op0=ALU.mult,
                op1=ALU.mult,
            )
            # h = Silu(x*rstd - mu*rstd), straight from PSUM, emitted fp16
            for t in range(NT):
                nc.scalar.activation(
                    out=h_sb[:, b, t * P : (t + 1) * P],
                    in_=psum_h[:, t * P : (t + 1) * P],
                    func=AF.Silu,
                    bias=negmr[:, t : t + 1],
                    scale=rstd[:, t : t + 1],
                )

        def main_phase(b, h_sb):
            n_acc = C if RECHUNK else NT
            accw = N if RECHUNK else C * P
            psvs = [
                ps_v.tile([P, 512], F32, name=f"psv{it}", tag="psv")
                for it in range(n_acc)
            ]
            for jc in range(JC):
                if MASK8:
                    evm_sb = p_evm.tile([P, 2 * C * N + N], U8)
                else:
                    evm_sb = p_evm.tile([P, EVW], F16)
                nc.sync.dma_start(out=evm_sb[:], in_=evm[b, jc * P : (jc + 1) * P])
                if MASK8:
                    ev16 = evm_sb[:, : 2 * C * N].bitcast(F16)
                    mk16 = p_mk.tile([P, N], F16)
                    nc.gpsimd.tensor_copy(
                        out=mk16[:], in_=evm_sb[:, 2 * C * N :].bitcast(F8)
                    )
                    mask_ap = mk16[:]
                else:
                    ev16 = evm_sb[:, : C * N]
                    mask_ap = evm_sb[:, C * N :]
                # mev[j,(c,i)] = ev[j,(c,i)] * mask[j,i]  (broadcast over c)
                if MODE == "no_dve":
                    mev = ev16.rearrange("p (c i) -> p c i", i=N)
                else:
                    mev_t = p_mev.tile([P, C, N], F16)
                    nc.vector.tensor_tensor(
                        out=mev_t[:],
                        in0=ev16.rearrange("p (c i) -> p c i", i=N),
                        in1=mask_ap.unsqueeze(1).broadcast_to((P, C, N)),
                        op=ALU.mult,
                    )
                    mev = mev_t[:]
                for k in range(n_acc):
                    # v[d, (c,il)] += sum_j h[j, d] * mev[j, (c, il)]
                    if RECHUNK:
                        rhs = mev[:, k, :]          # c-plane, 512 wide
                    else:
                        rhs = mev[:, :, k * P : (k + 1) * P]
                    nc.tensor.matmul(
                        out=psvs[k][:, :accw],
                        lhsT=h_sb[:, b, jc * P : (jc + 1) * P],
                        rhs=rhs,
                        start=(jc == 0),
                        stop=(jc == JC - 1),
                        skip_group_check=True,
                    )
            vout = p_vout.tile([P, n_acc, accw], F16)
            for k in range(n_acc):
                nc.scalar.activation(
                    out=vout[:, k, :], in_=psvs[k][:, :accw], func=AF.Copy
                )
                # store each chunk as soon as it's evicted, on the ACT HWDGE
                # ring so stores can't block loads on the SP-ring FIFO
                nc.scalar.dma_start(
                    out=out[b, :, k * accw : (k + 1) * accw], in_=vout[:, k, :]
                )

        def body():
            sT_sb = p_sT.tile([P, BL, N], F16)
            for b in range(BL):
                nc.sync.dma_start(out=sT_sb[:, b, :], in_=sT[b])
            if MODE == "dma_only":
                for b in range(BL):
                    for jc in range(JC):
                        if MASK8:
                            evm_sb = p_evm.tile([P, 2 * C * N + N], U8)
                        else:
                            evm_sb = p_evm.tile([P, EVW], F16)
                        nc.sync.dma_start(
                            out=evm_sb[:], in_=evm[b, jc * P : (jc + 1) * P]
                        )
                    nc.scalar.dma_start(out=out[b], in_=vz_c[:])
                return
            h_sb = p_h.tile([P, BL, N], F16)
            # software-pipeline: run the h-phase H_AHEAD batches ahead so its
            # long cross-engine LN chain resolves during earlier batches'
            # main phases instead of stalling DVE/PE at each batch boundary
            if H_AHEAD == 0:
                for b in range(BL):
                    h_phase(b, sT_sb, h_sb)
                    main_phase(b, h_sb)
            else:
                for b in range(min(H_AHEAD, BL)):
                    h_phase(b, sT_sb, h_sb)
                for b in range(BL):
                    main_phase(b, h_sb)
                    if b + H_AHEAD < BL:
                        h_phase(b + H_AHEAD, sT_sb, h_sb)

        n_loop, n_rem = divmod(reps, UNROLL)
        if n_loop > 0:
            with tc.For_i(0, n_loop, 1, staggered_reset=STAGGERED):
                for _ in range(UNROLL):
                    body()
        for _ in range(n_rem):
            body()

    _split_multi_waits(nc)
    return nc


_built_nc = None


def _get_nc():
    global _built_nc
    if _built_nc is None:
        _built_nc = build()
    return _built_nc


def shard_inputs(s, ev, mask, W1, b1):
    """Full inputs -> list of per-core input dicts (fp16 staged layouts)."""
    s = np.asarray(s, dtype=np.float32)
    ev = np.asarray(ev, dtype=np.float32)
    mask = np.asarray(mask, dtype=np.float32)
    W1 = np.asarray(W1, dtype=np.float32)
    b1 = np.asarray(b1, dtype=np.float32)
    w1t = np.ascontiguousarray(W1.T).astype(np.float16)
    w1rs = (W1.sum(axis=0) / H).reshape(H, 1).astype(np.float16)
    cst = np.concatenate(
        [
            np.ones((1, H), np.float32),
            np.tile(b1[None, :], (1, NT)),
            np.full((1, NT), float(b1.mean()), np.float32),
        ],
        axis=1,
    ).astype(np.float16)
    in_maps = []
    for m in range(NCORES):
        bs = slice(m * BL, (m + 1) * BL)
        evt = ev[bs].transpose(0, 2, 3, 1).reshape(BL, N, C * N)  # [b, j, (c,i)]
        mst = mask[bs, :, :, 0].transpose(0, 2, 1)                # [b, j, i]
        if MASK8:
            import ml_dtypes

            evm = np.concatenate(
                [
                    evt.astype(np.float16).view(np.uint8),
                    mst.astype(ml_dtypes.float8_e4m3).view(np.uint8),
                ],
                axis=2,
            )
        else:
            evm = np.concatenate([evt, mst], axis=2).astype(np.float16)
        in_maps.append(
            {
                "evm": np.ascontiguousarray(evm),
                "sT": np.ascontiguousarray(s[bs].transpose(0, 2, 1)).astype(
                    np.float16
                ),
                "w1t": w1t,
                "w1rs": w1rs,
                "cst": cst,
            }
        )
    return in_maps


def unshard_output(per_core_outs):
    """list of per-core "out" arrays [BL, H, NT*C*P] fp16 -> full (B, N, 3, H)."""
    parts = []
    for o in per_core_outs:
        o = o.astype(np.float32)
        if RECHUNK:
            o = o.reshape(BL, H, C, N).transpose(0, 3, 2, 1)
        else:
            o = o.reshape(BL, H, NT, C, P).transpose(0, 2, 4, 3, 1)
        parts.append(np.ascontiguousarray(o).reshape(BL, N, C, H))
    return np.concatenate(parts, axis=0)


_executor = None


def _get_executor():
    """Build the sharded PJRT executable once; reuse across kernel() calls."""
    global _executor
    if _executor is not None:
        return _executor
    import jax
    from jax.sharding import Mesh, PartitionSpec
    from jax.experimental.shard_map import shard_map

    from concourse import bass2jax

    bass2jax.install_neuronx_cc_hook()
    nc = _get_nc()
    partition_name = nc.partition_id_tensor.name if nc.partition_id_tensor else None
    in_names, out_names, out_avals, zero_outs = [], [], [], []
    for alloc in nc.m.functions[0].allocations:
        if not isinstance(alloc, mybir.MemoryLocationSet):
            continue
        name = alloc.memorylocations[0].name
        if alloc.kind == "ExternalInput":
            if name != partition_name:
                in_names.append(name)
        elif alloc.kind == "ExternalOutput":
            out_names.append(name)
            shape = tuple(alloc.tensor_shape)
            dtype = mybir.dt.np(alloc.dtype)
            out_avals.append(jax.core.ShapedArray(shape, dtype))
            zero_outs.append(np.zeros(shape, dtype))
    n_params = len(in_names)
    all_in_names = list(in_names) + list(out_names)
    if partition_name is not None:
        all_in_names.append(partition_name)

    def _body(*args):
        operands = list(args)
        if partition_name is not None:
            operands.append(bass2jax.partition_id_tensor())
        outs = bass2jax._bass_exec_p.bind(
            *operands,
            out_avals=tuple(out_avals),
            in_names=tuple(all_in_names),
            out_names=tuple(out_names),
            lowering_input_output_aliases=(),
            sim_require_finite=True,
            sim_require_nnan=True,
            nc=nc,
        )
        return tuple(outs)

    devices = jax.devices()[:NCORES]
    mesh = Mesh(np.asarray(devices), ("core",))
    donate = tuple(range(n_params, n_params + len(out_names)))
    fn = jax.jit(
        shard_map(
            _body,
            mesh=mesh,
            in_specs=(PartitionSpec("core"),) * (n_params + len(out_names)),
            out_specs=(PartitionSpec("core"),) * len(out_names),
            check_rep=False,
        ),
        donate_argnums=donate,
        keep_unused=True,
    )
    _executor = (fn, in_names, out_names, out_avals, zero_outs)
    return _executor


def kernel(s, ev, mask, W1, b1):
    fn, in_names, out_names, out_avals, zero_outs = _get_executor()
    in_maps = shard_inputs(s, ev, mask, W1, b1)
    concat_in = [
        np.concatenate([in_maps[c][nm] for c in range(NCORES)], axis=0)
        for nm in in_names
    ]
    concat_zeros = [
        np.zeros((NCORES * z.shape[0], *z.shape[1:]), z.dtype) for z in zero_outs
    ]
    out_arrs = fn(*concat_in, *concat_zeros)
    i = out_names.index("out")
    o = np.asarray(out_arrs[i]).reshape(NCORES, *out_avals[i].shape)
    return unshard_output([o[c] for c in range(NCORES)])
